# revision 1
# baseline (speedup 1.0000x reference)
"""Cross-attention kernel for Trainium2 (8 NeuronCores, SPMD data-parallel).

Problem: O = softmax(Q @ K^T) @ V with B=4, Lq=Lk=4096, D=64, fp32 (no
1/sqrt(d) scaling).

Sharding: 8 cores = 4 batches x 2 Lq-halves. Each core handles a
[2048, 64] Q shard against the full [4096, 64] K/V of its batch.
Independent outputs -> no collectives.

Per-core algorithm (layouts chosen so nothing is transposed on-chip):
  - Host supplies QT [64, 2048] / KT [64, 4096] in fp16 (D on partitions),
    duplicated on-chip across both partition halves so two k-chunks'
    score matmuls run concurrently in the PE array via row tiling
    (contraction is only 64 rows deep).
  - ST[k, q] = matmul(lhsT=KT chunk [64,128], rhs=QT [64,512]) -> PSUM.
  - PT = exp(ST) on the scalar engine, written as bf16 (no max
    subtraction: |scores| < ~50, exp fits fp32/bf16 range; fp16 P would
    underflow).  The scalar engine at 1 elem/cycle/lane is the kernel's
    bottleneck, so exp instructions are kept at 1024 free elements.
  - OT[65, q] += matmul(lhsT=VA chunk [128, 65] bf16, rhs=PT [128, 512]):
    VA = concat([V, ones], 1); rows 0..63 accumulate unnormalized output,
    row 64 the softmax denominator. PT is consumed directly as lhsT-free
    rhs - no transpose anywhere.
  - Normalize: fast-reciprocal of row 64, gpsimd partition-broadcast,
    multiply, DMA out OT [64, 2048]; host transposes back.
"""

import sys

for _p in ("/opt/trn_rl_repo", "/opt/pypackages"):
    if _p not in sys.path:
        sys.path.insert(0, _p)

from contextlib import ExitStack

import ml_dtypes
import numpy as np

import concourse.bacc as bacc
import concourse.mybir as mybir
import concourse.tile as tile
from concourse.bass_utils import run_bass_kernel_spmd

# Problem constants (hardcoded per contract).
B, LQ, LK, D = 4, 4096, 4096, 64
N_CORES = 8
LQ_SHARD = LQ * B // N_CORES  # 2048
QB = 1024  # q-block (exp instruction free-size; 2 PSUM banks)
NQB = LQ_SHARD // QB  # 2
KC = 128  # k-chunk (contraction tile for the PV matmul)
NKC = LK // KC  # 32
SL = 512  # matmul moving-dim slice (one PSUM bank)
NSL = QB // SL  # 2

F32 = mybir.dt.float32
F16 = mybir.dt.float16
BF16 = mybir.dt.bfloat16

BF16NP = ml_dtypes.bfloat16

PACK_S = True  # row-tile two k-chunks' score matmuls concurrently
FAST_RECIP = True  # approx+NR reciprocal (~2 ULP) instead of exact (~6.5us)

KT_PIECE = 512  # kt DMA piece width (cols); 4 k-chunks per piece
VA_PIECE = 8  # va DMA piece size in k-chunks


def _build_program():
    nc = bacc.Bacc(
        "TRN2",
        target_bir_lowering=False,
        debug=False,
        num_devices=N_CORES,
    )
    qt_d = nc.declare_dram_parameter("QT", [D, LQ_SHARD], F16, isOutput=False)
    kt_d = nc.declare_dram_parameter("KT", [D, LK], F16, isOutput=False)
    va_d = nc.declare_dram_parameter("VA", [LK, D + 1], BF16, isOutput=False)
    ot_d = nc.declare_dram_parameter("OT", [D, LQ_SHARD], F32, isOutput=True)

    with tile.TileContext(nc) as tc, ExitStack() as ctx:
        singles = ctx.enter_context(tc.tile_pool(name="singles", bufs=1))
        st_pool = ctx.enter_context(tc.tile_pool(name="st", bufs=2, space="PSUM"))
        ot_pool = ctx.enter_context(tc.tile_pool(name="ot", bufs=2, space="PSUM"))
        pt_pool = ctx.enter_context(tc.tile_pool(name="pt", bufs=3))
        out_pool = ctx.enter_context(tc.tile_pool(name="out", bufs=2))
        norm_pool = ctx.enter_context(tc.tile_pool(name="norm", bufs=4))

        # Preload the exp activation table while input DMAs run.
        warm = singles.tile([1, 2], F32)
        nc.vector.memset(warm[:, :], 0.0)
        nc.scalar.activation(
            out=warm[:, :], in_=warm[:, :],
            func=mybir.ActivationFunctionType.Exp,
        )

        # QT/KT duplicated across both partition halves for PE row tiling.
        # Inputs are split into halves (separate tiles) so the first score
        # matmuls don't wait for the full 2 MB of loads; keeping the piece
        # count low preserves the Tile scheduler's pairing of the row-tiled
        # matmuls (many small tiles reorder the PE stream and let HAM
        # re-throttle the PE clock).
        va_r = va_d[:, :].rearrange("(c p) d -> p c d", p=KC)
        KH = LK // 2  # kt half width
        VH = NKC // 2  # va half size in chunks
        kt_sb = []
        qt_sb = []
        va_sb = []
        for h in range(2):
            tq = singles.tile([2 * D, QB], F16, name=f"qt{h}")
            sq = slice(h * QB, (h + 1) * QB)
            nc.sync.dma_start(out=tq[0:D, :], in_=qt_d[:, sq])
            nc.sync.dma_start(out=tq[D : 2 * D, :], in_=qt_d[:, sq])
            qt_sb.append(tq)
            t = singles.tile([2 * D, KH], F16, name=f"kt{h}")
            sl = slice(h * KH, (h + 1) * KH)
            nc.sync.dma_start(out=t[0:D, :], in_=kt_d[:, sl])
            nc.sync.dma_start(out=t[D : 2 * D, :], in_=kt_d[:, sl])
            kt_sb.append(t)
            tv = singles.tile([KC, VH, D + 1], BF16, name=f"va{h}")
            nc.sync.dma_start(
                out=tv[:, :, :], in_=va_r[:, h * VH : (h + 1) * VH, :]
            )
            va_sb.append(tv)

        def kt_ap(half, c):
            # [64, 128] fp16 weights for chunk c from partition half `half`
            t = kt_sb[c * KC // KH]
            off = (c * KC) % KH
            return t[half * D : (half + 1) * D, off : off + KC]

        def va_ap(c):
            return va_sb[c // VH][:, c % VH, :]

        for qb in range(NQB):
            ot_ps = ot_pool.tile([D + 1, QB], F32)
            for cp in range(NKC // 2):  # chunk pairs, row-tiled in the PE
                c0, c1 = 2 * cp, 2 * cp + 1
                st_a = st_pool.tile([KC, QB], F32, tag="st")
                st_b = st_pool.tile([KC, QB], F32, tag="st")
                for s in range(NSL):
                    q0 = qb * QB + s * SL
                    qt = qt_sb[qb]
                    if PACK_S:
                        nc.tensor.matmul(
                            out=st_a[:, s * SL : (s + 1) * SL],
                            lhsT=kt_ap(0, c0),
                            rhs=qt[0:D, s * SL : (s + 1) * SL],
                            start=True,
                            stop=True,
                            tile_position=(0, 0),
                        )
                        nc.tensor.matmul(
                            out=st_b[:, s * SL : (s + 1) * SL],
                            lhsT=kt_ap(1, c1),
                            rhs=qt[D : 2 * D, s * SL : (s + 1) * SL],
                            start=True,
                            stop=True,
                            tile_position=(D, 0),
                        )
                    else:
                        nc.tensor.matmul(
                            out=st_a[:, s * SL : (s + 1) * SL],
                            lhsT=kt_ap(0, c0),
                            rhs=qt[0:D, s * SL : (s + 1) * SL],
                            start=True,
                            stop=True,
                        )
                        nc.tensor.matmul(
                            out=st_b[:, s * SL : (s + 1) * SL],
                            lhsT=kt_ap(0, c1),
                            rhs=qt[0:D, s * SL : (s + 1) * SL],
                            start=True,
                            stop=True,
                        )
                for c, st_ps in ((c0, st_a), (c1, st_b)):
                    pt = pt_pool.tile([KC, QB], BF16)
                    nc.scalar.activation(
                        out=pt[:, :],
                        in_=st_ps[:, :],
                        func=mybir.ActivationFunctionType.Exp,
                    )
                    for s in range(NSL):
                        nc.tensor.matmul(
                            out=ot_ps[:, s * SL : (s + 1) * SL],
                            lhsT=va_ap(c),
                            rhs=pt[:, s * SL : (s + 1) * SL],
                            start=(c == 0),
                            stop=(c == NKC - 1),
                        )
            # Normalize: O[d, q] = OT[d, q] / OT[64, q]
            recip = norm_pool.tile([1, QB], F32)
            if FAST_RECIP:
                den = norm_pool.tile([1, QB], F32)
                nc.vector.tensor_copy(den[:, :], ot_ps[D : D + 1, :])
                scratch = norm_pool.tile([1, QB], F32)
                nc.vector.reciprocal_approx_accurate(
                    recip[:, :], den[:, :], scratch[:, :]
                )
            else:
                nc.vector.reciprocal(out=recip[:, :], in_=ot_ps[D : D + 1, :])
            bcast = norm_pool.tile([D, QB], F32)
            nc.gpsimd.partition_broadcast(bcast[:, :], recip[:, :])
            o_sb = out_pool.tile([D, QB], F32)
            nc.vector.tensor_mul(o_sb[:, :], ot_ps[0:D, :], bcast[:, :])
            nc.sync.dma_start(
                out=ot_d[:, qb * QB : (qb + 1) * QB], in_=o_sb[:, :]
            )

    nc.finalize()
    return nc


_PROGRAM_CACHE = {}


def _get_program():
    if "nc" not in _PROGRAM_CACHE:
        _PROGRAM_CACHE["nc"] = _build_program()
    return _PROGRAM_CACHE["nc"]


def _make_in_maps(Q, K, V):
    Q = np.asarray(Q, dtype=np.float32)
    K = np.asarray(K, dtype=np.float32)
    V = np.asarray(V, dtype=np.float32)
    in_maps = []
    ones = np.ones((LK, 1), dtype=np.float32)
    for core in range(N_CORES):
        b, half = core // 2, core % 2
        q_shard = Q[b, half * LQ_SHARD : (half + 1) * LQ_SHARD, :]  # [2048, 64]
        qt = np.ascontiguousarray(q_shard.T).astype(np.float16)  # [64, 2048]
        kt = np.ascontiguousarray(K[b].T).astype(np.float16)  # [64, 4096]
        va = np.concatenate([V[b], ones], axis=1).astype(BF16NP)  # [4096, 65]
        in_maps.append({"QT": qt, "KT": kt, "VA": np.ascontiguousarray(va)})
    return in_maps


def _run(Q, K, V, trace=False, **spmd_kwargs):
    nc = _get_program()
    in_maps = _make_in_maps(Q, K, V)
    res = run_bass_kernel_spmd(
        nc, in_maps, list(range(N_CORES)), trace=trace, **spmd_kwargs
    )
    out = np.empty((B, LQ, D), dtype=np.float32)
    for core in range(N_CORES):
        b, half = core // 2, core % 2
        ot = res.results[core]["OT"]  # [64, 2048]
        out[b, half * LQ_SHARD : (half + 1) * LQ_SHARD, :] = ot.T
    return out, res


def kernel(Q, K, V):
    out, _ = _run(Q, K, V, trace=False)
    return out



# revision 2
# speedup vs baseline: 1.5238x; 1.5238x over previous
"""Cross-attention kernel for Trainium2 (8 NeuronCores, SPMD data-parallel).

Problem: O = softmax(Q @ K^T) @ V with B=4, Lq=Lk=4096, D=64, fp32 (no
1/sqrt(d) scaling).

Sharding: 8 cores = 4 batches x 2 Lq-halves. Each core handles a
[2048, 64] Q shard against the full [4096, 64] K/V of its batch.
Independent outputs -> no collectives.

Per-core algorithm (layouts chosen so nothing is transposed on-chip):
  - Host supplies QT [128, 2048] fp16 (D on partitions, duplicated into
    rows 64..127), KT [128, 4096] fp16 with rows 64..127 ZERO, and
    VA [4096, 128] bf16 = [V | ones | zeros].
  - The zero padding makes every matmul a full 128x128-activity op.
    TRN2's PE_HAM clock gate only un-throttles (1.2 -> 2.4 GHz) when the
    PE array's activity is high; half-array matmuls (contraction 64, or
    65 output partitions) never cross the threshold and the whole kernel
    runs at half clock. Padded operands cost the same cycles (cycles =
    moving-dim size) but register full activity -> warm clock.
  - ST[k, q] = matmul(lhsT=KTpad chunk [128,128], rhs=QT [128,512]);
    rows 64..127 of KTpad are zero so the duplicated QT rows contribute 0.
  - PT = exp(ST) on the scalar engine, written as bf16 (no max
    subtraction: |scores| < ~50, exp fits fp32/bf16 range; fp16 P would
    underflow). The scalar engine at 1 elem/cycle/lane is the kernel's
    bottleneck, so exp instructions are kept at 1024 free elements.
  - OT[128, q] += matmul(lhsT=VA chunk [128, 128] bf16, rhs=PT [128, 512]):
    rows 0..63 accumulate unnormalized output, row 64 the softmax
    denominator, rows 65..127 zeros. PT is consumed directly as rhs -
    no transpose anywhere.
  - Normalize: fast-reciprocal of row 64, gpsimd partition-broadcast,
    multiply, DMA out OT [64, 2048]; host transposes back.
"""

import sys

for _p in ("/opt/trn_rl_repo", "/opt/pypackages"):
    if _p not in sys.path:
        sys.path.insert(0, _p)

from contextlib import ExitStack

import ml_dtypes
import numpy as np

import concourse.bacc as bacc
import concourse.mybir as mybir
import concourse.tile as tile
from concourse.bass_utils import run_bass_kernel_spmd

# Problem constants (hardcoded per contract).
B, LQ, LK, D = 4, 4096, 4096, 64
N_CORES = 8
LQ_SHARD = LQ * B // N_CORES  # 2048
QB = 1024  # q-block (exp instruction free-size; 2 PSUM banks)
NQB = LQ_SHARD // QB  # 2
KC = 128  # k-chunk (contraction tile for the PV matmul)
NKC = LK // KC  # 32
SL = 512  # matmul moving-dim slice (one PSUM bank)
NSL = QB // SL  # 2

F32 = mybir.dt.float32
F16 = mybir.dt.float16
BF16 = mybir.dt.bfloat16

BF16NP = ml_dtypes.bfloat16

FAST_RECIP = True  # approx+NR reciprocal (~2 ULP) instead of exact (~6.5us)

KT_PIECE = 512  # kt DMA piece width (cols); 4 k-chunks per piece
VA_PIECE = 8  # va DMA piece size in k-chunks


def _build_program():
    nc = bacc.Bacc(
        "TRN2",
        target_bir_lowering=False,
        debug=False,
        num_devices=N_CORES,
    )
    qt_d = nc.declare_dram_parameter("QT", [2 * D, LQ_SHARD], F16, isOutput=False)
    kt_d = nc.declare_dram_parameter("KT", [2 * D, LK], F16, isOutput=False)
    va_d = nc.declare_dram_parameter("VA", [LK, KC], BF16, isOutput=False)
    ot_d = nc.declare_dram_parameter("OT", [D, LQ_SHARD], F32, isOutput=True)

    with tile.TileContext(nc) as tc, ExitStack() as ctx:
        singles = ctx.enter_context(tc.tile_pool(name="singles", bufs=1))
        st_pool = ctx.enter_context(tc.tile_pool(name="st", bufs=2, space="PSUM"))
        ot_pool = ctx.enter_context(tc.tile_pool(name="ot", bufs=2, space="PSUM"))
        pt_pool = ctx.enter_context(tc.tile_pool(name="pt", bufs=3))
        out_pool = ctx.enter_context(tc.tile_pool(name="out", bufs=2))
        norm_pool = ctx.enter_context(tc.tile_pool(name="norm", bufs=4))

        # Preload the exp activation table while input DMAs run.
        warm = singles.tile([1, 2], F32)
        nc.vector.memset(warm[:, :], 0.0)
        nc.scalar.activation(
            out=warm[:, :], in_=warm[:, :],
            func=mybir.ActivationFunctionType.Exp,
        )

        # Inputs are split into pieces so the first score matmuls don't
        # wait for the full 2.5 MB of loads.
        va_r = va_d[:, :].rearrange("(c p) d -> p c d", p=KC)
        KH = LK // 2  # kt half width
        VH = NKC // 2  # va half size in chunks
        kt_sb = []
        qt_sb = []
        va_sb = []
        for h in range(2):
            tq = singles.tile([2 * D, QB], F16, name=f"qt{h}")
            sq = slice(h * QB, (h + 1) * QB)
            nc.sync.dma_start(out=tq[:, :], in_=qt_d[:, sq])
            qt_sb.append(tq)
            t = singles.tile([2 * D, KH], F16, name=f"kt{h}")
            for p in range(KH // KT_PIECE):
                sl = slice(p * KT_PIECE, (p + 1) * KT_PIECE)
                sg = slice(h * KH + p * KT_PIECE, h * KH + (p + 1) * KT_PIECE)
                nc.sync.dma_start(out=t[:, sl], in_=kt_d[:, sg])
            kt_sb.append(t)
            tv = singles.tile([KC, VH, KC], BF16, name=f"va{h}")
            for p in range(VH // VA_PIECE):
                sl = slice(p * VA_PIECE, (p + 1) * VA_PIECE)
                sg = slice(h * VH + p * VA_PIECE, h * VH + (p + 1) * VA_PIECE)
                nc.sync.dma_start(out=tv[:, sl, :], in_=va_r[:, sg, :])
            va_sb.append(tv)

        def kt_ap(c):
            # [128, 128] fp16 weights for chunk c (rows 64..127 zero)
            t = kt_sb[c * KC // KH]
            off = (c * KC) % KH
            return t[:, off : off + KC]

        def va_ap(c):
            return va_sb[c // VH][:, c % VH, :]

        for qb in range(NQB):
            ot_ps = ot_pool.tile([KC, QB], F32)
            qt = qt_sb[qb]
            for c in range(NKC):
                st_ps = st_pool.tile([KC, QB], F32, tag="st")
                for s in range(NSL):
                    nc.tensor.matmul(
                        out=st_ps[:, s * SL : (s + 1) * SL],
                        lhsT=kt_ap(c),
                        rhs=qt[:, s * SL : (s + 1) * SL],
                        start=True,
                        stop=True,
                    )
                pt = pt_pool.tile([KC, QB], BF16)
                nc.scalar.activation(
                    out=pt[:, :],
                    in_=st_ps[:, :],
                    func=mybir.ActivationFunctionType.Exp,
                )
                for s in range(NSL):
                    nc.tensor.matmul(
                        out=ot_ps[:, s * SL : (s + 1) * SL],
                        lhsT=va_ap(c),
                        rhs=pt[:, s * SL : (s + 1) * SL],
                        start=(c == 0),
                        stop=(c == NKC - 1),
                    )
            # Normalize: O[d, q] = OT[d, q] / OT[64, q]
            recip = norm_pool.tile([1, QB], F32)
            if FAST_RECIP:
                den = norm_pool.tile([1, QB], F32)
                nc.vector.tensor_copy(den[:, :], ot_ps[D : D + 1, :])
                scratch = norm_pool.tile([1, QB], F32)
                nc.vector.reciprocal_approx_accurate(
                    recip[:, :], den[:, :], scratch[:, :]
                )
            else:
                nc.vector.reciprocal(out=recip[:, :], in_=ot_ps[D : D + 1, :])
            bcast = norm_pool.tile([D, QB], F32)
            nc.gpsimd.partition_broadcast(bcast[:, :], recip[:, :])
            o_sb = out_pool.tile([D, QB], F32)
            nc.vector.tensor_mul(o_sb[:, :], ot_ps[0:D, :], bcast[:, :])
            nc.sync.dma_start(
                out=ot_d[:, qb * QB : (qb + 1) * QB], in_=o_sb[:, :]
            )

    nc.finalize()
    return nc


_PROGRAM_CACHE = {}


def _get_program():
    if "nc" not in _PROGRAM_CACHE:
        _PROGRAM_CACHE["nc"] = _build_program()
    return _PROGRAM_CACHE["nc"]


def _make_in_maps(Q, K, V):
    Q = np.asarray(Q, dtype=np.float32)
    K = np.asarray(K, dtype=np.float32)
    V = np.asarray(V, dtype=np.float32)
    in_maps = []
    for core in range(N_CORES):
        b, half = core // 2, core % 2
        q_shard = Q[b, half * LQ_SHARD : (half + 1) * LQ_SHARD, :]  # [2048, 64]
        qt1 = q_shard.T.astype(np.float16)  # [64, 2048]
        qt = np.concatenate([qt1, qt1], axis=0)  # [128, 2048] (dup rows)
        kt = np.zeros((2 * D, LK), dtype=np.float16)  # [128, 4096]
        kt[:D, :] = K[b].T.astype(np.float16)
        va = np.zeros((LK, KC), dtype=BF16NP)  # [4096, 128]
        va[:, :D] = V[b].astype(BF16NP)
        va[:, D] = 1.0
        in_maps.append(
            {
                "QT": np.ascontiguousarray(qt),
                "KT": np.ascontiguousarray(kt),
                "VA": np.ascontiguousarray(va),
            }
        )
    return in_maps


def _run(Q, K, V, trace=False, **spmd_kwargs):
    nc = _get_program()
    in_maps = _make_in_maps(Q, K, V)
    res = run_bass_kernel_spmd(
        nc, in_maps, list(range(N_CORES)), trace=trace, **spmd_kwargs
    )
    out = np.empty((B, LQ, D), dtype=np.float32)
    for core in range(N_CORES):
        b, half = core // 2, core % 2
        ot = res.results[core]["OT"]  # [64, 2048]
        out[b, half * LQ_SHARD : (half + 1) * LQ_SHARD, :] = ot.T
    return out, res


def kernel(Q, K, V):
    out, _ = _run(Q, K, V, trace=False)
    return out
